# revision 56
# baseline (speedup 1.0000x reference)
"""Trainium2 Bass kernel for the Preisach hysteresis scan (nn_BaseHysteresis).

Math: the per-step relay update is affine in the state:
    s_t = a_t * s_{t-1} + sigma_t * (1 - a_t)
with a_t = sigmoid((alpha - h_t)/temp) on up-steps (sigma=+1) and
     a_t = sigmoid((h_t - beta)/temp)  on down-steps (sigma=-1).
Substituting e_t = s_t - sigma_t gives
    e_t = a_t * (e_{t-1} + delta_t),  delta_t = sigma_{t-1} - sigma_t
which is exactly the hardware tensor_tensor_scan form
    state = (data0 + state) * data1   (op0=add, op1=mult)
The sigmoid argument mixes a per-hysteron term and a per-timestep term:
    X[p,t] = w_p * m_t + v_p + c_t
      w = (alpha+beta)/temp, v = -beta/temp, m_t = [up], c_t = -sigma_t*h_t/temp
built on the tensor engine as a K=6 matmul with bf16 hi/lo splits of w, v, c
(PSUM accumulates in fp32 so the split keeps ~1e-3 absolute accuracy).
The weighted reduction m_t = sum_i d_i s_i,t = sum_i d_i e_i,t + sigma_t*sum d
is a K=128 PE dot product per 128-hysteron group, accumulated in PSUM.

Sharding: hysteron dim across the 8 cores (2513 per core, padded to 20*128);
host combines the 8 partial dot products and applies the final affine.
"""

import numpy as np
import ml_dtypes

TEMP = 0.01
T = 4096
N_MESH = 20100
NCORES = 8
P = 128
G = 20                 # hysteron groups of 128 per core
NPC = P * G            # padded hysterons per core (2560)
PER_CORE = -(-N_MESH // NCORES)  # 2513
SUPER = 2048
NSUPER = T // SUPER
BLK = 512
NBLK = SUPER // BLK
ABLK = 512                  # ACT/X-psum chunk
NABLK = SUPER // ABLK

_CACHE = {}


def _build_nc():
    import concourse.tile as tile
    from concourse import bacc, mybir

    f32 = mybir.dt.float32
    f32r = mybir.dt.float32r
    bf16 = mybir.dt.bfloat16
    Alu = mybir.AluOpType
    Act = mybir.ActivationFunctionType

    nc = bacc.Bacc("TRN2", target_bir_lowering=False, debug=False)

    xw = nc.dram_tensor("xw", [6, T], bf16, kind="ExternalInput")
    lhs = nc.dram_tensor("lhs", [6, NPC], bf16, kind="ExternalInput")
    dcol = nc.dram_tensor("dcol", [P, G], f32r, kind="ExternalInput")
    dbc = nc.dram_tensor("dbc", [P, T], f32, kind="ExternalInput")
    ones = nc.dram_tensor("ones", [P, 1], f32r, kind="ExternalInput")
    pm = nc.dram_tensor("pm", [1, T], f32, kind="ExternalOutput")
    dsum = nc.dram_tensor("dsum", [1, 1], f32, kind="ExternalOutput")

    with tile.TileContext(nc) as tc:
        with (
            tc.tile_pool(name="const", bufs=1) as constp,
            tc.tile_pool(name="apool", bufs=3) as apool,
            tc.tile_pool(name="epool", bufs=3) as epool,
            tc.tile_pool(name="pxp", bufs=3, space="PSUM") as pxp,
            tc.tile_pool(name="pmp", bufs=1, space="PSUM") as pmp,
            tc.tile_pool(name="dsp", bufs=1, space="PSUM") as dsp,
            tc.tile_pool(name="outp", bufs=2) as outp,
        ):
            xw_sb = constp.tile([6, T], bf16, tag="xw")
            lhs_sb = constp.tile([6, NPC], bf16, tag="lhs")
            dcol_sb = constp.tile([P, G], f32r, tag="dcol")
            dbc_sb = constp.tile([P, T], f32, tag="dbc")
            ones_sb = constp.tile([P, 1], f32r, tag="ones")
            carry = [
                constp.tile([P, 1], f32r, tag=f"carry{g}", name=f"carry{g}")
                for g in range(G)
            ]

            # the first scan is gated on dbc block 0 — queue it first, fine
            # grained so it lands early; later blocks are staggered below
            bcast_done = set()

            def bcast_dbc(b):
                if b in bcast_done or b >= T // BLK:
                    return
                bcast_done.add(b)
                nc.sync.dma_start(
                    dbc_sb[:, b * BLK:(b + 1) * BLK],
                    dbc[:, b * BLK:(b + 1) * BLK],
                )

            # first slivers of xw/lhs land first so the (0,0) X-matmul chain
            # starts before the bulk transfers finish
            nc.sync.dma_start(xw_sb[:, 0:BLK], xw[:, 0:BLK])
            nc.sync.dma_start(lhs_sb[:, 0:P], lhs[:, 0:P])
            bcast_dbc(0)
            nc.sync.dma_start(xw_sb[:, BLK:T], xw[:, BLK:T])
            nc.sync.dma_start(lhs_sb[:, P:NPC], lhs[:, P:NPC])
            bcast_dbc(1)
            nc.sync.dma_start(dcol_sb[:], dcol[:])
            nc.sync.dma_start(ones_sb[:], ones[:])

            # density sum (independent of scans — runs early, own PSUM bank)
            ds_ps = dsp.tile([1, G], f32, tag="ds")
            nc.tensor.matmul(
                ds_ps[:], lhsT=ones_sb[:], rhs=dcol_sb[:], start=True, stop=True
            )
            ds_sb = outp.tile([1, 1], f32, tag="ds_sb")
            nc.vector.tensor_reduce(
                ds_sb[:], ds_ps[:], axis=mybir.AxisListType.X, op=Alu.add
            )
            nc.sync.dma_start(dsum[:], ds_sb[:])

            def emit_pm_out(s_of, pm_tile, b):
                pm_sb = outp.tile(
                    [1, BLK], f32, tag="pm_sb", bufs=4, name=f"pm_sb_{s_of}_{b}",
                )
                nc.scalar.copy(pm_sb[:], pm_tile[:, b * BLK:(b + 1) * BLK])
                nc.sync.dma_start(
                    pm[0:1, s_of * SUPER + b * BLK: s_of * SUPER + (b + 1) * BLK],
                    pm_sb[:],
                )

            def produce_a(s, g):
                a_t = apool.tile([P, SUPER], f32, tag="a", name=f"a_{s}_{g}")
                for ab in range(NABLK):
                    px = pxp.tile([P, ABLK], f32, tag="px", name=f"px_{s}_{g}_{ab}")
                    # finer sub-chunks for the very first block so the
                    # first scan piece starts as early as possible
                    subs = [256, 256] if (s == 0 and g == 0 and ab == 0) else [ABLK]
                    lo = 0
                    for slen in subs:
                        hi = lo + slen
                        nc.tensor.matmul(
                            px[:, lo:hi],
                            lhsT=lhs_sb[:, g * P:(g + 1) * P],
                            rhs=xw_sb[
                                :,
                                s * SUPER + ab * ABLK + lo:
                                s * SUPER + ab * ABLK + hi,
                            ],
                            start=True,
                            stop=True,
                        )
                        nc.scalar.activation(
                            a_t[:, ab * ABLK + lo: ab * ABLK + hi],
                            px[:, lo:hi],
                            Act.Sigmoid,
                        )
                        lo = hi
                return a_t

            pending_pm = None
            a_next = None
            for s in range(NSUPER):
                pm_ps = pmp.tile([1, SUPER], f32, tag="pmps", name=f"pm_ps{s}")
                for g in range(G):
                    if s == 0 and g >= 1:
                        bcast_dbc(NABLK - 1 + g)  # blocks for super 1, staggered
                    if a_next is not None:
                        a_t = a_next
                        a_next = None
                    else:
                        a_t = produce_a(s, g)
                    e_t = epool.tile([P, SUPER], f32r, tag="e", name=f"e_{s}_{g}")
                    split_first = s == 0 and g == 0
                    split_last = s == NSUPER - 1 and g == G - 1
                    if split_first or split_last:
                        # split scans: first so it starts as soon as the first
                        # A columns exist; last so tail m-matmuls overlap
                        init0 = 0.0 if s == 0 else carry[g][:]
                        pieces = (
                            [256, 256, 512, 1024] if split_first
                            else [512] * NABLK
                        )
                        lo = 0
                        for plen in pieces:
                            hi = lo + plen
                            if s == 0:
                                bcast_dbc(lo // BLK)
                                bcast_dbc((hi - 1) // BLK)
                            nc.vector.tensor_tensor_scan(
                                e_t[:, lo:hi],
                                dbc_sb[:, s * SUPER + lo: s * SUPER + hi],
                                a_t[:, lo:hi],
                                init0 if lo == 0 else e_t[:, lo - 1: lo],
                                Alu.add,
                                Alu.mult,
                            )
                            lo = hi
                    else:
                        init = 0.0 if s == 0 else carry[g][:]
                        nc.vector.tensor_tensor_scan(
                            e_t[:],
                            dbc_sb[:, s * SUPER:(s + 1) * SUPER],
                            a_t[:],
                            init,
                            Alu.add,
                            Alu.mult,
                        )
                    if s < NSUPER - 1:
                        nc.sync.dma_start(carry[g][:], e_t[:, SUPER - 1: SUPER])
                    if pending_pm is not None:
                        # previous super's PSUM->SBUF->DRAM copies, deferred so
                        # this super's first sigmoids won the ACT at the boundary
                        ps_of, ps_tile = pending_pm
                        for b in range(NBLK):
                            emit_pm_out(ps_of, ps_tile, b)
                        pending_pm = None
                    for b in range(NBLK):
                        nc.tensor.matmul(
                            pm_ps[:, b * BLK:(b + 1) * BLK],
                            lhsT=dcol_sb[:, g: g + 1],
                            rhs=e_t[:, b * BLK:(b + 1) * BLK],
                            start=(g == 0),
                            stop=(g == G - 1),
                        )
                        if g == G - 1 and s == NSUPER - 1:
                            emit_pm_out(s, pm_ps, b)
                if s < NSUPER - 1:
                    pending_pm = (s, pm_ps)

    nc.finalize()
    return nc


def _get_nc():
    if "nc" not in _CACHE:
        _CACHE["nc"] = _build_nc()
    return _CACHE["nc"]


def _bf16_split(x):
    hi = x.astype(ml_dtypes.bfloat16)
    lo = (x - hi.astype(np.float64)).astype(ml_dtypes.bfloat16)
    return hi, lo


def kernel(h, mesh_points, density, scale, offset, slope, _trace=False):
    from concourse.bass_utils import run_bass_kernel_spmd

    h = np.asarray(h, np.float32)
    mesh_points = np.asarray(mesh_points, np.float32)
    density = np.asarray(density, np.float32)

    # ---- host prep of per-timestep scalars (O(T)) ----
    hprev = np.concatenate(([np.float32(0.0)], h[:-1]))
    up = h > hprev
    sig = np.where(up, 1.0, -1.0)
    h64 = h.astype(np.float64)
    c = -sig * h64 / TEMP
    mrow = np.where(up, 1.0, 0.0)
    delta = np.empty(T)
    delta[0] = -1.0 - sig[0]
    delta[1:] = sig[:-1] - sig[1:]

    c_hi, c_lo = _bf16_split(c)
    one_row = np.ones(T, ml_dtypes.bfloat16)
    xw_host = np.stack(
        [mrow.astype(ml_dtypes.bfloat16), mrow.astype(ml_dtypes.bfloat16),
         one_row, one_row, c_hi, c_lo]
    )
    dbc_host = np.ascontiguousarray(
        np.broadcast_to(delta.astype(np.float32)[None, :], (P, T))
    )
    ones_host = np.ones((P, 1), np.float32)

    # ---- per-core hysteron shards (O(N)) ----
    beta = mesh_points[:, 0].astype(np.float64)
    alpha = mesh_points[:, 1].astype(np.float64)
    w = (alpha + beta) / TEMP
    v = -beta / TEMP
    w_hi, w_lo = _bf16_split(w)
    v_hi, v_lo = _bf16_split(v)

    in_maps = []
    for cidx in range(NCORES):
        lo_i = cidx * PER_CORE
        hi_i = min(N_MESH, lo_i + PER_CORE)
        n = hi_i - lo_i

        def pad(x, fill=0):
            out = np.full(NPC, fill, x.dtype)
            out[:n] = x[lo_i:hi_i]
            return out

        ones_col = np.ones(NPC, ml_dtypes.bfloat16)
        ones_col[n:] = 0
        lhs_host = np.stack(
            [pad(w_hi), pad(w_lo), pad(v_hi), pad(v_lo), ones_col, ones_col]
        )
        d_pad = pad(density)
        dcol_host = np.ascontiguousarray(d_pad.reshape(G, P).T)

        in_maps.append(
            {
                "xw": xw_host,
                "lhs": lhs_host,
                "dcol": dcol_host,
                "dbc": dbc_host,
                "ones": ones_host,
            }
        )

    nc = _get_nc()
    res = run_bass_kernel_spmd(
        nc, in_maps, core_ids=list(range(NCORES)), trace=_trace
    )
    _CACHE["last_results"] = res

    pm_total = np.zeros(T, np.float64)
    d_total = 0.0
    for cidx in range(NCORES):
        pm_total += res.results[cidx]["pm"][0].astype(np.float64)
        d_total += float(res.results[cidx]["dsum"][0, 0])

    m = (pm_total + sig * d_total) / d_total
    out = (
        float(np.asarray(scale).reshape(-1)[0]) * m
        + float(np.asarray(offset).reshape(-1)[0])
        + h64 * float(np.asarray(slope).reshape(-1)[0])
    )
    return out.astype(np.float32)



# revision 57
# speedup vs baseline: 1.2116x; 1.2116x over previous
"""Trainium2 Bass kernel for the Preisach hysteresis scan (nn_BaseHysteresis).

Math: the per-step relay update is affine in the state:
    s_t = a_t * s_{t-1} + sigma_t * (1 - a_t)
with a_t = sigmoid((alpha - h_t)/temp) on up-steps (sigma=+1) and
     a_t = sigmoid((h_t - beta)/temp)  on down-steps (sigma=-1).
Substituting e_t = s_t - sigma_t gives
    e_t = a_t * (e_{t-1} + delta_t),  delta_t = sigma_{t-1} - sigma_t
which is exactly the hardware tensor_tensor_scan form
    state = (data0 + state) * data1   (op0=add, op1=mult)
The sigmoid argument mixes a per-hysteron term and a per-timestep term:
    X[p,t] = w_p * m_t + v_p + c_t
      w = (alpha+beta)/temp, v = -beta/temp, m_t = [up], c_t = -sigma_t*h_t/temp
built on the tensor engine as a K=6 matmul with bf16 hi/lo splits of w, v, c
(PSUM accumulates in fp32 so the split keeps ~1e-3 absolute accuracy).
The weighted reduction m_t = sum_i d_i s_i,t = sum_i d_i e_i,t + sigma_t*sum d
is a K=128 PE dot product per 128-hysteron group, accumulated in PSUM.

Sharding: hysteron dim across the 8 cores (2513 per core, padded to 20*128);
host combines the 8 partial dot products and applies the final affine.
"""

import numpy as np
import ml_dtypes

TEMP = 0.01
T = 4096
N_MESH = 20100
NCORES = 8
P = 128
G = 20                 # hysteron groups of 128 per core
NPC = P * G            # padded hysterons per core (2560)
PER_CORE = -(-N_MESH // NCORES)  # 2513
SUPER = 2048
NSUPER = T // SUPER
BLK = 512
NBLK = SUPER // BLK
ABLK = 512                  # ACT/X-psum chunk
NABLK = SUPER // ABLK

_CACHE = {}


def _build_nc():
    import concourse.tile as tile
    from concourse import bacc, mybir

    f32 = mybir.dt.float32
    f32r = mybir.dt.float32r
    bf16 = mybir.dt.bfloat16
    Alu = mybir.AluOpType
    Act = mybir.ActivationFunctionType

    nc = bacc.Bacc("TRN2", target_bir_lowering=False, debug=False)

    xw = nc.dram_tensor("xw", [6, T], bf16, kind="ExternalInput")
    lhs = nc.dram_tensor("lhs", [6, NPC], bf16, kind="ExternalInput")
    dcol = nc.dram_tensor("dcol", [P, G], f32r, kind="ExternalInput")
    dbc = nc.dram_tensor("dbc", [P, T], f32, kind="ExternalInput")
    ones = nc.dram_tensor("ones", [P, 1], f32r, kind="ExternalInput")
    pm = nc.dram_tensor("pm", [1, T], f32, kind="ExternalOutput")
    dsum = nc.dram_tensor("dsum", [1, 1], f32, kind="ExternalOutput")

    with tile.TileContext(nc) as tc:
        with (
            tc.tile_pool(name="const", bufs=1) as constp,
            tc.tile_pool(name="apool", bufs=3) as apool,
            tc.tile_pool(name="epool", bufs=3) as epool,
            tc.tile_pool(name="pxp", bufs=3, space="PSUM") as pxp,
            tc.tile_pool(name="pmp", bufs=1, space="PSUM") as pmp,
            tc.tile_pool(name="dsp", bufs=1, space="PSUM") as dsp,
            tc.tile_pool(name="outp", bufs=2) as outp,
        ):
            xw_sb = constp.tile([6, T], bf16, tag="xw")
            lhs_sb = constp.tile([6, NPC], bf16, tag="lhs")
            dcol_sb = constp.tile([P, G], f32r, tag="dcol")
            dbc_sb = constp.tile([P, T], f32, tag="dbc")
            ones_sb = constp.tile([P, 1], f32r, tag="ones")
            carry = [
                constp.tile([P, 1], f32r, tag=f"carry{g}", name=f"carry{g}")
                for g in range(G)
            ]

            # the first scan is gated on dbc block 0 — queue it first, fine
            # grained so it lands early; later blocks are staggered below
            bcast_done = set()

            def bcast_dbc(b):
                if b in bcast_done or b >= T // BLK:
                    return
                bcast_done.add(b)
                nc.sync.dma_start(
                    dbc_sb[:, b * BLK:(b + 1) * BLK],
                    dbc[:, b * BLK:(b + 1) * BLK],
                )

            nc.sync.dma_start(xw_sb[:], xw[:])
            nc.sync.dma_start(lhs_sb[:], lhs[:])
            bcast_dbc(0)
            bcast_dbc(1)
            nc.sync.dma_start(dcol_sb[:], dcol[:])
            nc.sync.dma_start(ones_sb[:], ones[:])

            # density sum (independent of scans — runs early, own PSUM bank)
            ds_ps = dsp.tile([1, G], f32, tag="ds")
            nc.tensor.matmul(
                ds_ps[:], lhsT=ones_sb[:], rhs=dcol_sb[:], start=True, stop=True
            )
            ds_sb = outp.tile([1, 1], f32, tag="ds_sb")
            nc.vector.tensor_reduce(
                ds_sb[:], ds_ps[:], axis=mybir.AxisListType.X, op=Alu.add
            )
            nc.sync.dma_start(dsum[:], ds_sb[:])

            def emit_pm_out(s_of, pm_tile, b):
                pm_sb = outp.tile(
                    [1, BLK], f32, tag="pm_sb", bufs=4, name=f"pm_sb_{s_of}_{b}",
                )
                nc.scalar.copy(pm_sb[:], pm_tile[:, b * BLK:(b + 1) * BLK])
                nc.sync.dma_start(
                    pm[0:1, s_of * SUPER + b * BLK: s_of * SUPER + (b + 1) * BLK],
                    pm_sb[:],
                )

            def produce_a(s, g):
                a_t = apool.tile([P, SUPER], f32, tag="a", name=f"a_{s}_{g}")
                for ab in range(NABLK):
                    px = pxp.tile([P, ABLK], f32, tag="px", name=f"px_{s}_{g}_{ab}")
                    # finer sub-chunks for the very first block so the
                    # first scan piece starts as early as possible
                    subs = [256, 256] if (s == 0 and g == 0 and ab == 0) else [ABLK]
                    lo = 0
                    for slen in subs:
                        hi = lo + slen
                        nc.tensor.matmul(
                            px[:, lo:hi],
                            lhsT=lhs_sb[:, g * P:(g + 1) * P],
                            rhs=xw_sb[
                                :,
                                s * SUPER + ab * ABLK + lo:
                                s * SUPER + ab * ABLK + hi,
                            ],
                            start=True,
                            stop=True,
                        )
                        nc.scalar.activation(
                            a_t[:, ab * ABLK + lo: ab * ABLK + hi],
                            px[:, lo:hi],
                            Act.Sigmoid,
                        )
                        lo = hi
                return a_t

            pending_pm = None
            a_next = None
            for s in range(NSUPER):
                pm_ps = pmp.tile([1, SUPER], f32, tag="pmps", name=f"pm_ps{s}")
                for g in range(G):
                    if s == 0 and g >= 1:
                        bcast_dbc(NABLK - 1 + g)  # blocks for super 1, staggered
                    if a_next is not None:
                        a_t = a_next
                        a_next = None
                    else:
                        a_t = produce_a(s, g)
                    e_t = epool.tile([P, SUPER], f32r, tag="e", name=f"e_{s}_{g}")
                    split_first = s == 0 and g == 0
                    split_last = s == NSUPER - 1 and g == G - 1
                    if split_first or split_last:
                        # split scans: first so it starts as soon as the first
                        # A columns exist; last so tail m-matmuls overlap
                        init0 = 0.0 if s == 0 else carry[g][:]
                        pieces = (
                            [256, 256, 512, 1024] if split_first
                            else [512] * NABLK
                        )
                        lo = 0
                        for plen in pieces:
                            hi = lo + plen
                            if s == 0:
                                bcast_dbc(lo // BLK)
                                bcast_dbc((hi - 1) // BLK)
                            nc.vector.tensor_tensor_scan(
                                e_t[:, lo:hi],
                                dbc_sb[:, s * SUPER + lo: s * SUPER + hi],
                                a_t[:, lo:hi],
                                init0 if lo == 0 else e_t[:, lo - 1: lo],
                                Alu.add,
                                Alu.mult,
                            )
                            lo = hi
                    else:
                        init = 0.0 if s == 0 else carry[g][:]
                        nc.vector.tensor_tensor_scan(
                            e_t[:],
                            dbc_sb[:, s * SUPER:(s + 1) * SUPER],
                            a_t[:],
                            init,
                            Alu.add,
                            Alu.mult,
                        )
                    if s < NSUPER - 1:
                        nc.sync.dma_start(carry[g][:], e_t[:, SUPER - 1: SUPER])
                    if pending_pm is not None:
                        # previous super's PSUM->SBUF->DRAM copies, deferred so
                        # this super's first sigmoids won the ACT at the boundary
                        ps_of, ps_tile = pending_pm
                        for b in range(NBLK):
                            emit_pm_out(ps_of, ps_tile, b)
                        pending_pm = None
                    for b in range(NBLK):
                        nc.tensor.matmul(
                            pm_ps[:, b * BLK:(b + 1) * BLK],
                            lhsT=dcol_sb[:, g: g + 1],
                            rhs=e_t[:, b * BLK:(b + 1) * BLK],
                            start=(g == 0),
                            stop=(g == G - 1),
                        )
                        if g == G - 1 and s == NSUPER - 1:
                            emit_pm_out(s, pm_ps, b)
                if s < NSUPER - 1:
                    pending_pm = (s, pm_ps)

    nc.finalize()
    return nc


def _get_nc():
    if "nc" not in _CACHE:
        _CACHE["nc"] = _build_nc()
    return _CACHE["nc"]


def _bf16_split(x):
    hi = x.astype(ml_dtypes.bfloat16)
    lo = (x - hi.astype(np.float64)).astype(ml_dtypes.bfloat16)
    return hi, lo


def kernel(h, mesh_points, density, scale, offset, slope, _trace=False):
    from concourse.bass_utils import run_bass_kernel_spmd

    h = np.asarray(h, np.float32)
    mesh_points = np.asarray(mesh_points, np.float32)
    density = np.asarray(density, np.float32)

    # ---- host prep of per-timestep scalars (O(T)) ----
    hprev = np.concatenate(([np.float32(0.0)], h[:-1]))
    up = h > hprev
    sig = np.where(up, 1.0, -1.0)
    h64 = h.astype(np.float64)
    c = -sig * h64 / TEMP
    mrow = np.where(up, 1.0, 0.0)
    delta = np.empty(T)
    delta[0] = -1.0 - sig[0]
    delta[1:] = sig[:-1] - sig[1:]

    c_hi, c_lo = _bf16_split(c)
    one_row = np.ones(T, ml_dtypes.bfloat16)
    xw_host = np.stack(
        [mrow.astype(ml_dtypes.bfloat16), mrow.astype(ml_dtypes.bfloat16),
         one_row, one_row, c_hi, c_lo]
    )
    dbc_host = np.ascontiguousarray(
        np.broadcast_to(delta.astype(np.float32)[None, :], (P, T))
    )
    ones_host = np.ones((P, 1), np.float32)

    # ---- per-core hysteron shards (O(N)) ----
    beta = mesh_points[:, 0].astype(np.float64)
    alpha = mesh_points[:, 1].astype(np.float64)
    w = (alpha + beta) / TEMP
    v = -beta / TEMP
    w_hi, w_lo = _bf16_split(w)
    v_hi, v_lo = _bf16_split(v)

    in_maps = []
    for cidx in range(NCORES):
        lo_i = cidx * PER_CORE
        hi_i = min(N_MESH, lo_i + PER_CORE)
        n = hi_i - lo_i

        def pad(x, fill=0):
            out = np.full(NPC, fill, x.dtype)
            out[:n] = x[lo_i:hi_i]
            return out

        ones_col = np.ones(NPC, ml_dtypes.bfloat16)
        ones_col[n:] = 0
        lhs_host = np.stack(
            [pad(w_hi), pad(w_lo), pad(v_hi), pad(v_lo), ones_col, ones_col]
        )
        d_pad = pad(density)
        dcol_host = np.ascontiguousarray(d_pad.reshape(G, P).T)

        in_maps.append(
            {
                "xw": xw_host,
                "lhs": lhs_host,
                "dcol": dcol_host,
                "dbc": dbc_host,
                "ones": ones_host,
            }
        )

    nc = _get_nc()
    res = run_bass_kernel_spmd(
        nc, in_maps, core_ids=list(range(NCORES)), trace=_trace
    )
    _CACHE["last_results"] = res

    pm_total = np.zeros(T, np.float64)
    d_total = 0.0
    for cidx in range(NCORES):
        pm_total += res.results[cidx]["pm"][0].astype(np.float64)
        d_total += float(res.results[cidx]["dsum"][0, 0])

    m = (pm_total + sig * d_total) / d_total
    out = (
        float(np.asarray(scale).reshape(-1)[0]) * m
        + float(np.asarray(offset).reshape(-1)[0])
        + h64 * float(np.asarray(slope).reshape(-1)[0])
    )
    return out.astype(np.float32)



# revision 64
# speedup vs baseline: 1.2134x; 1.0015x over previous
"""Trainium2 Bass kernel for the Preisach hysteresis scan (nn_BaseHysteresis).

Math: the per-step relay update is affine in the state:
    s_t = a_t * s_{t-1} + sigma_t * (1 - a_t)
with a_t = sigmoid((alpha - h_t)/temp) on up-steps (sigma=+1) and
     a_t = sigmoid((h_t - beta)/temp)  on down-steps (sigma=-1).
Substituting e_t = s_t - sigma_t gives
    e_t = a_t * (e_{t-1} + delta_t),  delta_t = sigma_{t-1} - sigma_t
which is exactly the hardware tensor_tensor_scan form
    state = (data0 + state) * data1   (op0=add, op1=mult)
The sigmoid argument mixes a per-hysteron term and a per-timestep term:
    X[p,t] = w_p * m_t + v_p + c_t
      w = (alpha+beta)/temp, v = -beta/temp, m_t = [up], c_t = -sigma_t*h_t/temp
built on the tensor engine as a K=6 matmul with bf16 hi/lo splits of w, v, c
(PSUM accumulates in fp32 so the split keeps ~1e-3 absolute accuracy).
The weighted reduction m_t = sum_i d_i s_i,t = sum_i d_i e_i,t + sigma_t*sum d
is a K=128 PE dot product per 128-hysteron group, accumulated in PSUM.

Sharding: hysteron dim across the 8 cores (2513 per core, padded to 20*128);
host combines the 8 partial dot products and applies the final affine.
"""

import numpy as np
import ml_dtypes

TEMP = 0.01
T = 4096
N_MESH = 20100
NCORES = 8
P = 128
G = 20                 # hysteron groups of 128 per core
NPC = P * G            # padded hysterons per core (2560)
PER_CORE = -(-N_MESH // NCORES)  # 2513
SUPER = 2048
NSUPER = T // SUPER
BLK = 512
NBLK = SUPER // BLK
ABLK = 512                  # ACT/X-psum chunk
NABLK = SUPER // ABLK

_CACHE = {}


def _build_nc():
    import concourse.tile as tile
    from concourse import bacc, mybir

    f32 = mybir.dt.float32
    f32r = mybir.dt.float32r
    bf16 = mybir.dt.bfloat16
    Alu = mybir.AluOpType
    Act = mybir.ActivationFunctionType

    nc = bacc.Bacc("TRN2", target_bir_lowering=False, debug=False)

    xw = nc.dram_tensor("xw", [6, T], bf16, kind="ExternalInput")
    lhs = nc.dram_tensor("lhs", [6, NPC], bf16, kind="ExternalInput")
    dcol = nc.dram_tensor("dcol", [P, G], f32r, kind="ExternalInput")
    dbc = nc.dram_tensor("dbc", [P, T], f32, kind="ExternalInput")
    pm = nc.dram_tensor("pm", [1, T], f32, kind="ExternalOutput")

    with tile.TileContext(nc) as tc:
        with (
            tc.tile_pool(name="const", bufs=1) as constp,
            tc.tile_pool(name="apool", bufs=3) as apool,
            tc.tile_pool(name="epool", bufs=3) as epool,
            tc.tile_pool(name="pxp", bufs=3, space="PSUM") as pxp,
            tc.tile_pool(name="pmp", bufs=1, space="PSUM") as pmp,
            tc.tile_pool(name="outp", bufs=2) as outp,
        ):
            xw_sb = constp.tile([6, T], bf16, tag="xw")
            lhs_sb = constp.tile([6, NPC], bf16, tag="lhs")
            dcol_sb = constp.tile([P, G], f32r, tag="dcol")
            dbc_sb = constp.tile([P, T], f32, tag="dbc")
            carry = [
                constp.tile([P, 1], f32r, tag=f"carry{g}", name=f"carry{g}")
                for g in range(G)
            ]

            # the first scan is gated on dbc block 0 — queue it first, fine
            # grained so it lands early; later blocks are staggered below
            bcast_done = set()

            def bcast_dbc(b):
                if b in bcast_done or b >= T // BLK:
                    return
                bcast_done.add(b)
                nc.sync.dma_start(
                    dbc_sb[:, b * BLK:(b + 1) * BLK],
                    dbc[:, b * BLK:(b + 1) * BLK],
                )

            nc.sync.dma_start(xw_sb[:], xw[:])
            nc.sync.dma_start(lhs_sb[:], lhs[:])
            bcast_dbc(0)
            bcast_dbc(1)
            nc.sync.dma_start(dcol_sb[:], dcol[:])

            def emit_pm_out(s_of, pm_tile, b):
                pm_sb = outp.tile(
                    [1, BLK], f32, tag="pm_sb", bufs=4, name=f"pm_sb_{s_of}_{b}",
                )
                nc.scalar.copy(pm_sb[:], pm_tile[:, b * BLK:(b + 1) * BLK])
                nc.sync.dma_start(
                    pm[0:1, s_of * SUPER + b * BLK: s_of * SUPER + (b + 1) * BLK],
                    pm_sb[:],
                )

            def produce_a(s, g):
                a_t = apool.tile([P, SUPER], f32, tag="a", name=f"a_{s}_{g}")
                for ab in range(NABLK):
                    px = pxp.tile([P, ABLK], f32, tag="px", name=f"px_{s}_{g}_{ab}")
                    # finer sub-chunks for the very first block so the
                    # first scan piece starts as early as possible
                    subs = [256, 256] if (s == 0 and g == 0 and ab == 0) else [ABLK]
                    lo = 0
                    for slen in subs:
                        hi = lo + slen
                        nc.tensor.matmul(
                            px[:, lo:hi],
                            lhsT=lhs_sb[:, g * P:(g + 1) * P],
                            rhs=xw_sb[
                                :,
                                s * SUPER + ab * ABLK + lo:
                                s * SUPER + ab * ABLK + hi,
                            ],
                            start=True,
                            stop=True,
                        )
                        nc.scalar.activation(
                            a_t[:, ab * ABLK + lo: ab * ABLK + hi],
                            px[:, lo:hi],
                            Act.Sigmoid,
                        )
                        lo = hi
                return a_t

            pending_pm = None
            a_next = None
            for s in range(NSUPER):
                pm_ps = pmp.tile([1, SUPER], f32, tag="pmps", name=f"pm_ps{s}")
                for g in range(G):
                    if s == 0 and g >= 1:
                        bcast_dbc(NABLK - 1 + g)  # blocks for super 1, staggered
                    if a_next is not None:
                        a_t = a_next
                        a_next = None
                    else:
                        a_t = produce_a(s, g)
                    e_t = epool.tile([P, SUPER], f32r, tag="e", name=f"e_{s}_{g}")
                    split_first = s == 0 and g == 0
                    split_last = s == NSUPER - 1 and g == G - 1
                    if split_first or split_last:
                        # split scans: first so it starts as soon as the first
                        # A columns exist; last so tail m-matmuls overlap
                        init0 = 0.0 if s == 0 else carry[g][:]
                        pieces = (
                            [256, 256, 512, 1024] if split_first
                            else [512] * NABLK
                        )
                        lo = 0
                        for plen in pieces:
                            hi = lo + plen
                            if s == 0:
                                bcast_dbc(lo // BLK)
                                bcast_dbc((hi - 1) // BLK)
                            nc.vector.tensor_tensor_scan(
                                e_t[:, lo:hi],
                                dbc_sb[:, s * SUPER + lo: s * SUPER + hi],
                                a_t[:, lo:hi],
                                init0 if lo == 0 else e_t[:, lo - 1: lo],
                                Alu.add,
                                Alu.mult,
                            )
                            lo = hi
                    else:
                        init = 0.0 if s == 0 else carry[g][:]
                        nc.vector.tensor_tensor_scan(
                            e_t[:],
                            dbc_sb[:, s * SUPER:(s + 1) * SUPER],
                            a_t[:],
                            init,
                            Alu.add,
                            Alu.mult,
                        )
                    if s < NSUPER - 1:
                        nc.sync.dma_start(carry[g][:], e_t[:, SUPER - 1: SUPER])
                    if pending_pm is not None:
                        # previous super's PSUM->SBUF->DRAM copies, deferred so
                        # this super's first sigmoids won the ACT at the boundary
                        ps_of, ps_tile = pending_pm
                        for b in range(NBLK):
                            emit_pm_out(ps_of, ps_tile, b)
                        pending_pm = None
                    for b in range(NBLK):
                        nc.tensor.matmul(
                            pm_ps[:, b * BLK:(b + 1) * BLK],
                            lhsT=dcol_sb[:, g: g + 1],
                            rhs=e_t[:, b * BLK:(b + 1) * BLK],
                            start=(g == 0),
                            stop=(g == G - 1),
                        )
                        if g == G - 1 and s == NSUPER - 1:
                            emit_pm_out(s, pm_ps, b)
                if s < NSUPER - 1:
                    pending_pm = (s, pm_ps)

    nc.finalize()
    return nc


def _get_nc():
    if "nc" not in _CACHE:
        _CACHE["nc"] = _build_nc()
    return _CACHE["nc"]


def _bf16_split(x):
    hi = x.astype(ml_dtypes.bfloat16)
    lo = (x - hi.astype(np.float64)).astype(ml_dtypes.bfloat16)
    return hi, lo


def kernel(h, mesh_points, density, scale, offset, slope, _trace=False):
    from concourse.bass_utils import run_bass_kernel_spmd

    h = np.asarray(h, np.float32)
    mesh_points = np.asarray(mesh_points, np.float32)
    density = np.asarray(density, np.float32)

    # ---- host prep of per-timestep scalars (O(T)) ----
    hprev = np.concatenate(([np.float32(0.0)], h[:-1]))
    up = h > hprev
    sig = np.where(up, 1.0, -1.0)
    h64 = h.astype(np.float64)
    c = -sig * h64 / TEMP
    mrow = np.where(up, 1.0, 0.0)
    delta = np.empty(T)
    delta[0] = -1.0 - sig[0]
    delta[1:] = sig[:-1] - sig[1:]

    c_hi, c_lo = _bf16_split(c)
    one_row = np.ones(T, ml_dtypes.bfloat16)
    xw_host = np.stack(
        [mrow.astype(ml_dtypes.bfloat16), mrow.astype(ml_dtypes.bfloat16),
         one_row, one_row, c_hi, c_lo]
    )
    dbc_host = np.ascontiguousarray(
        np.broadcast_to(delta.astype(np.float32)[None, :], (P, T))
    )

    # ---- per-core hysteron shards (O(N)) ----
    beta = mesh_points[:, 0].astype(np.float64)
    alpha = mesh_points[:, 1].astype(np.float64)
    w = (alpha + beta) / TEMP
    v = -beta / TEMP
    w_hi, w_lo = _bf16_split(w)
    v_hi, v_lo = _bf16_split(v)

    in_maps = []
    for cidx in range(NCORES):
        lo_i = cidx * PER_CORE
        hi_i = min(N_MESH, lo_i + PER_CORE)
        n = hi_i - lo_i

        def pad(x, fill=0):
            out = np.full(NPC, fill, x.dtype)
            out[:n] = x[lo_i:hi_i]
            return out

        ones_col = np.ones(NPC, ml_dtypes.bfloat16)
        ones_col[n:] = 0
        lhs_host = np.stack(
            [pad(w_hi), pad(w_lo), pad(v_hi), pad(v_lo), ones_col, ones_col]
        )
        d_pad = pad(density)
        dcol_host = np.ascontiguousarray(d_pad.reshape(G, P).T)

        in_maps.append(
            {
                "xw": xw_host,
                "lhs": lhs_host,
                "dcol": dcol_host,
                "dbc": dbc_host,
            }
        )

    nc = _get_nc()
    res = run_bass_kernel_spmd(
        nc, in_maps, core_ids=list(range(NCORES)), trace=_trace
    )
    _CACHE["last_results"] = res

    pm_total = np.zeros(T, np.float64)
    for cidx in range(NCORES):
        pm_total += res.results[cidx]["pm"][0].astype(np.float64)
    d_total = float(density.astype(np.float64).sum())

    m = (pm_total + sig * d_total) / d_total
    out = (
        float(np.asarray(scale).reshape(-1)[0]) * m
        + float(np.asarray(offset).reshape(-1)[0])
        + h64 * float(np.asarray(slope).reshape(-1)[0])
    )
    return out.astype(np.float32)

